# revision 35
# baseline (speedup 1.0000x reference)
"""GAT attention layer (gnn_message_passing) on 8 TRN2 NeuronCores.

Math (reference):
    h = inp @ W;  s1 = h @ a1;  s2 = h @ a2
    e = leaky_relu(s1 + s2^T, 0.2);  scores = where(adj>0, e, -9e15)
    out = elu(softmax_row(scores) @ h)

Device algorithm (per core, rows R = N/8), "rank-1 max" design:
  Softmax without max-subtraction; masked entries contribute exactly 0:
      out[i,:] = elu( (sum_j P[i,j] h[j,:]) / (sum_j P[i,j]) )
  with P = adj * e_eff and the row-constant exp(0.2*s1[i]) dropped:
      e_eff[i,j] = exp(lrelu(s1[i]+s2[j]) - 0.2*s1[i])
                 = max(exp(0.8*s1[i] + s2[j]), exp(0.2*s2[j]))
                 = max(u1[i] * u2[j], v2[j])
  u1 = exp(0.8*s1) rides a partition-broadcast row vector; u2, v2 are
  per-j (per-partition) scalars exponentiated from the fused stage-1
  matmul's extra columns.  One DVE tensor_scalar (mult + max, 2x) per
  chunk produces e_eff — no ACT exp pass and no DVE relu pass over the
  N x R score matrix at all.  The adjacency mask is stored fp8 {0,1} in
  DRAM (half the HBM bytes of bf16), upcast to bf16 by a GpSimd
  cast-DMA (prefetchable, off every compute engine's critical path) and
  applied with one pair-wide DVE tensor_tensor multiply (2x mode).

  Everything is in the TRANSPOSED orientation [j (partitions), i (free)]
  so the attention matmul needs no on-chip transposes.  num|denom in one
  bf16 matmul: rhs = [h | ones] (257 cols), lhsT = P^T slices.

  Own-row s1 values come from the main stage-1 pass: the host rolls the
  j-axis per core so chunks 0..OC-1 are exactly the core's own rows
  (attention contraction order is permutation-invariant).  The first two
  input tiles are front-loaded on the DMA queue ahead of the weights so
  the PE starts ~8us earlier.

Host-side work is layout/dtype only: slicing, transposition, rolling and
fp32->bf16/fp8 casts of inputs. All FLOPs happen on device.
"""
import sys

sys.path.insert(0, "/opt/trn_rl_repo")

import numpy as np
import ml_dtypes

import concourse.bass as bass
import concourse.mybir as mybir
from concourse.tile import TileContext
from concourse.bass_utils import run_bass_kernel_spmd

F32 = mybir.dt.float32
BF16 = mybir.dt.bfloat16
F16 = mybir.dt.float16
F8 = mybir.dt.float8e4
AF = mybir.ActivationFunctionType
ALU = mybir.AluOpType

ALPHA = 0.2
N_CORES = 8


# ---------------------------------------------------------------------------
# walrus workaround: this build rejects >1 inline sync-wait per instruction
# ("Too many sync wait commands"); move the excess into same-engine NoOps.
# ---------------------------------------------------------------------------
def split_excess_waits(nc, nop_capacity=1):
    counter = 0
    for f in nc.m.functions:
        for bb in f.blocks:
            out = []
            changed = False
            for inst in bb.instructions:
                si = inst.sync_info
                max_inline = 0 if isinstance(inst, mybir.InstDrain) else 1
                if si is not None and len(si.on_wait) > max_inline:
                    waits = list(si.on_wait)
                    if max_inline:
                        excess, keep = waits[:-max_inline], waits[-max_inline:]
                    else:
                        excess, keep = waits, []
                    for s in range(0, len(excess), nop_capacity):
                        counter += 1
                        nop = mybir.InstNoOp(
                            name=f"WSPLIT-{counter}", ins=[], outs=[]
                        )
                        nop.engine = inst.engine
                        nop.sync_info = mybir.SyncInfo(
                            on_wait=excess[s:s + nop_capacity], on_update=[]
                        )
                        out.append(nop)
                    inst.sync_info = mybir.SyncInfo(
                        on_wait=keep, on_update=list(si.on_update)
                    )
                    changed = True
                out.append(inst)
            if changed:
                bb.instructions = out


# ---------------------------------------------------------------------------
# kernel builder
# ---------------------------------------------------------------------------
def build_nc(NJ, R, IN, OUT, acc_banks=6, lead2=7, bufs_inp=4, bufs_adj=8,
             bufs_e=8, bufs_pt=10, bufs_ep=3, split_waits=True,
             s1_mod=1, hcopy_dve_mod=0, gps_tt_mod=0, QW=2):
    P = 128
    KC = IN // P          # contraction chunks for inp @ W
    JC = NJ // P          # j (column/source-node) chunks
    NJC2 = JC // 2        # jc pairs (stage-1 granularity)
    NQ = JC // QW         # stage-2 mask-tile granularity
    OC = R // P           # own-row chunks
    G = -(-OC // acc_banks)   # row groups so live accumulators <= acc_banks
    OCG = OC // G
    assert OCG * G == OC
    ISPAN = OCG * P       # free width of the transposed score tiles
    E = OUT + 3           # W | w2 | 0.2*w2 | w1 columns of the fused rhs
    assert OC <= 2 * lead2, "own-row chunks must complete before stage-2"

    nc = bass.Bass()
    # Pre-tiled masks [128, G*NJC2*2*ISPAN]: pair (g, jc2) is the contiguous
    # col block (g*NJC2 + jc2)*2*ISPAN, laid out [b, i].
    # mcast: {0, 1}   — cast-DMA'd to bf16 for the DVE (rank-1 STT) pairs.
    # madd:  {0, -96} — accum-added into pre-exp scores for the ACT pairs.
    mcast = nc.declare_dram_parameter(
        "mcast", [P, (NJ * R) // P], F8, isOutput=False)
    madd = nc.declare_dram_parameter(
        "madd", [P, (NJ * R) // P], F8, isOutput=False)
    # inpT_t: [128, JC*IN] tiled so chunk (jc,kc) is cols jc*IN+kc*P..+P
    inpT_t = nc.declare_dram_parameter(
        "inpT_t", [P, JC * IN], BF16, isOutput=False)
    W = nc.declare_dram_parameter("W", [IN, OUT], BF16, isOutput=False)
    WT = nc.declare_dram_parameter("WT", [OUT, IN], BF16, isOutput=False)
    a12 = nc.declare_dram_parameter("a12", [OUT, 2], BF16, isOutput=False)
    ident = nc.declare_dram_parameter("ident", [P, P], F32, isOutput=False)
    out_own = nc.declare_dram_parameter("out", [R, OUT], F32, isOutput=True)
    s1_dram = nc.dram_tensor("s1_scratch", [R], F32)

    CC = OUT // P         # chunks of the OUT dim (for W^T @ a12)

    with TileContext(nc) as tc:
        with (
            tc.tile_pool(name="const", bufs=1) as constp,
            tc.tile_pool(name="wts", bufs=1) as wts,
            tc.tile_pool(name="hpool", bufs=1) as hpool,
            tc.tile_pool(name="s1p", bufs=1) as s1p,
            tc.tile_pool(name="inp_t", bufs=bufs_inp) as inp_p,
            tc.tile_pool(name="adjp", bufs=bufs_adj) as adjp,
            tc.tile_pool(name="eep", bufs=bufs_e) as eep,
            tc.tile_pool(name="ptp", bufs=bufs_pt) as ptp,
            tc.tile_pool(name="ep", bufs=bufs_ep) as ep,
            tc.tile_pool(name="pmisc", bufs=2, space="PSUM") as pmisc,
            tc.tile_pool(name="pacc", bufs=1, space="PSUM") as pacc,
        ):
            # ---- PE clock warm-up ----
            # ~24 dependency-free dummy matmuls (uninitialized SBUF, output
            # never read) keep the PE busy from t=0 so the HAM clock gate
            # opens (1.2 -> 2.4 GHz) before the first real matmul issues.
            warm_sb = constp.tile([P, P], BF16, tag="warm")
            nc.gpsimd.memset(warm_sb[:, :], 1.0)
            warm_ps = pmisc.tile([P, E], F32, tag="pm")
            for _ in range(36):
                nc.tensor.matmul(
                    warm_ps[:, 0:P], warm_sb[:, :], warm_sb[:, :],
                    start=True, stop=True,
                )
            # ---- stage 0: weights ----
            # Front-load the first two it2 input tiles so stage-1 matmuls
            # can start ~8us earlier than with weights first on the queue.
            it2_head = []
            for mi in range(2):
                t = inp_p.tile([P, 2 * IN], BF16, tag="it", name="it")
                nc.sync.dma_start(
                    out=t[:, :], in_=inpT_t[:, mi * 2 * IN:(mi + 1) * 2 * IN])
                it2_head.append(t)
            # rhs_cat[kc] = [W rows | w2 | 0.2*w2 | w1]  (bf16, 259 cols)
            rhs_cat = []
            for kc in range(KC):
                t = wts.tile([P, E], BF16, tag=f"rhsc{kc}", name=f"rhsc{kc}")
                nc.sync.dma_start(
                    out=t[:, 0:OUT], in_=W[kc * P:(kc + 1) * P, :]
                )
                rhs_cat.append(t)
            wt_sb = []
            a12_sb = []
            for cc in range(CC):
                t = wts.tile([P, IN], BF16, tag=f"wt{cc}", name=f"wt{cc}")
                nc.sync.dma_start(out=t[:, :], in_=WT[cc * P:(cc + 1) * P, :])
                wt_sb.append(t)
                t2 = wts.tile([P, 2], BF16, tag=f"a12_{cc}", name=f"a12_{cc}")
                nc.sync.dma_start(out=t2[:, :], in_=a12[cc * P:(cc + 1) * P, :])
                a12_sb.append(t2)
            ident_sb = constp.tile([P, P], F32, tag="ident")
            nc.sync.dma_start(out=ident_sb[:, :], in_=ident[:, :])
            # w12[k, 0:2] = (W @ [a1 a2])[k]  via  WT-blocks^T @ a12-blocks
            for kc in range(KC):
                pw = pmisc.tile([P, E], F32, tag="pm")
                for cc in range(CC):
                    nc.tensor.matmul(
                        pw[:, 0:2],
                        wt_sb[cc][:, kc * P:(kc + 1) * P],
                        a12_sb[cc][:, :],
                        start=(cc == 0),
                        stop=(cc == CC - 1),
                    )
                nc.vector.tensor_copy(rhs_cat[kc][:, OUT:OUT + 1], pw[:, 1:2])
                nc.vector.tensor_scalar_mul(
                    rhs_cat[kc][:, OUT + 1:OUT + 2], pw[:, 1:2], ALPHA)
                nc.vector.tensor_copy(
                    rhs_cat[kc][:, OUT + 2:OUT + 3], pw[:, 0:1])

            # ---- stage 1 bodies (fused h | s2-exps | s1 capture) ----
            h_sb = [None] * JC
            uv_sb = [None] * JC
            s1_stage = s1p.tile([P, OC], F32, tag="s1stage")

            def stage1_body(jc, it2, a):
                ph = pmisc.tile([P, E], F32, tag="pm")
                for kc in range(KC):
                    nc.tensor.matmul(
                        ph[:, :],
                        it2[:, a * IN + kc * P:a * IN + (kc + 1) * P],
                        rhs_cat[kc][:, :],
                        start=(kc == 0),
                        stop=(kc == KC - 1),
                    )
                # stage s-scalars out of PSUM fast (DVE), releasing ph
                # without waiting on the ACT queue; uv exps read the staging.
                uv = hpool.tile([P, 4], F32, tag=f"uv{jc}", name=f"uv{jc}")
                nc.scalar.copy(uv[:, 2:4], ph[:, OUT:OUT + 2])
                h = hpool.tile([P, OUT + 1], BF16, tag=f"h{jc}",
                               name=f"h{jc}")
                if hcopy_dve_mod and jc % hcopy_dve_mod == 0:
                    nc.vector.tensor_copy(h[:, 0:OUT], ph[:, 0:OUT])
                else:
                    nc.scalar.copy(h[:, 0:OUT], ph[:, 0:OUT])
                nc.gpsimd.memset(h[:, OUT:OUT + 1], 1.0)
                # u2 = exp(s2), v2 = exp(0.2*s2) from the staged raw values
                nc.scalar.activation(
                    uv[:, 0:2], uv[:, 2:4], AF.Exp, scale=1.0)
                h_sb[jc] = h
                uv_sb[jc] = uv
                if jc < OC:
                    nc.vector.tensor_copy(
                        s1_stage[:, jc:jc + 1], ph[:, OUT + 2:OUT + 3])

            # ---- stage 2 (attention pass) over jc pairs ----
            # Two balanced per-pair pipelines (both give
            # pT = adj * max(exp(0.8s1+s2), exp(0.2s2))):
            #   S3 (default): xm = max(0.8*s1 + s2, 0.2*s2)  fp16 [DVE TS 2x]
            #                 xm += {0,-96} mask     [GpSimd accum-add DMA]
            #                 pT = Exp(xm)                   [ACT, pair-wide]
            #   S1 (every s1_mod-th pair, rank-1 exp on DVE):
            #                 ee = (u1*u2) max v2            [DVE TS 2x]
            #                 pT = ee * adj{0,1}             [DVE TT 2x]
            def stage2_quad(g, gsl, q, accs):
                off = (g * NQ + q) * QW * ISPAN
                pt2 = ptp.tile([P, QW * ISPAN], BF16, tag="pt")
                at2 = adjp.tile([P, QW * ISPAN], BF16, tag="at")
                nc.gpsimd.dma_start(
                    out=at2[:, :], in_=mcast[:, off:off + QW * ISPAN],
                )
                ee2 = eep.tile([P, QW * ISPAN], BF16, tag="ee")
                for a in range(QW):
                    uv = uv_sb[q * QW + a]
                    nc.vector.tensor_scalar(
                        ee2[:, a * ISPAN:(a + 1) * ISPAN],
                        u1bc[:, gsl], uv[:, 0:1], uv[:, 1:2],
                        op0=ALU.mult, op1=ALU.max,
                    )
                nc.vector.tensor_tensor(
                    pt2[:, :], ee2[:, :], at2[:, :], op=ALU.mult)
                for a in range(QW):
                    jc = q * QW + a
                    rh = h_sb[jc][:, 0:OUT + 1]
                    for m in range(OCG):
                        nc.tensor.matmul(
                            accs[m][:, :],
                            pt2[:, a * ISPAN + m * P:a * ISPAN + (m + 1) * P],
                            rh,
                            start=(jc == 0),
                            stop=(jc == JC - 1),
                        )

            def epilogue(g, accs):
                for m in range(OCG):
                    acc = accs[m]
                    r = ep.tile([P, 1], F32, tag="r")
                    nc.vector.reciprocal(r[:, :], acc[:, OUT:OUT + 1])
                    t = ep.tile([P, OUT], F32, tag="t")
                    nc.vector.tensor_scalar_mul(t[:, :], acc[:, 0:OUT], r[:, :])
                    # elu = relu(t) + (exp(-relu(-t)) - 1)
                    rn = ep.tile([P, OUT], F32, tag="rn")
                    en = ep.tile([P, OUT], F32, tag="en")
                    ps = ep.tile([P, OUT], F32, tag="ps")
                    nc.scalar.activation(
                        rn[:, :], t[:, :], AF.Relu, scale=-1.0)
                    nc.scalar.activation(
                        en[:, :], rn[:, :], AF.Exp, scale=-1.0)
                    nc.scalar.activation(ps[:, :], t[:, :], AF.Relu)
                    res = ep.tile([P, OUT], F32, tag="res")
                    nc.vector.scalar_tensor_tensor(
                        res[:, :], ps[:, :], -1.0, en[:, :],
                        op0=ALU.add, op1=ALU.add,
                    )
                    row = (g * OCG + m) * P
                    nc.sync.dma_start(
                        out=out_own[row:row + P, :], in_=res[:, :]
                    )

            # ---- fused stage-1 + attention pass for g=0 ----
            gsl0 = slice(0, ISPAN)
            accs0 = [
                pacc.tile([P, OUT + 1], F32, tag=f"acc{m}", name=f"acc{m}")
                for m in range(OCG)
            ]
            u1bc = None
            for mi in range(max(NJC2, lead2 + (NQ - 1) * (QW // 2) + 1)):
                if mi < NJC2:
                    if mi < len(it2_head):
                        it2 = it2_head[mi]
                    else:
                        it2 = inp_p.tile([P, 2 * IN], BF16, tag="it",
                                         name="it")
                        nc.sync.dma_start(
                            out=it2[:, :],
                            in_=inpT_t[:, mi * 2 * IN:(mi + 1) * 2 * IN],
                        )
                    for a in range(2):
                        stage1_body(mi * 2 + a, it2, a)
                if mi == (OC + 1) // 2 - 1:
                    # own-row chunks staged: build the broadcast u1 vector
                    pt1 = pmisc.tile([P, E], F32, tag="pm")
                    nc.tensor.matmul(
                        pt1[:OC, 0:P], s1_stage[:, :], ident_sb[:, :],
                        is_transpose=True,
                    )
                    s1rows = s1p.tile([P, P], F32, tag="s1rows")
                    nc.vector.tensor_copy(s1rows[:OC, :], pt1[:OC, 0:P])
                    nc.sync.dma_start(
                        out=s1_dram[:].rearrange("(a b) -> a b", b=P),
                        in_=s1rows[:OC, :],
                    )
                    s1bc = s1p.tile([P, R], F32, tag="s1bc")
                    nc.sync.dma_start(
                        out=s1bc[:, :], in_=s1_dram[:].partition_broadcast(P)
                    )
                    u1bc = s1p.tile([P, R], BF16, tag="u1bc")
                    nc.scalar.activation(
                        u1bc[:, :], s1bc[:, :], AF.Exp, scale=1.0 - ALPHA)
                if mi >= lead2 and (mi - lead2) % (QW // 2) == 0:
                    q = (mi - lead2) // (QW // 2)
                    if q < NQ:
                        stage2_quad(0, gsl0, q, accs0)
            epilogue(0, accs0)

            # ---- remaining groups: pure attention passes ----
            for g in range(1, G):
                gsl = slice(g * ISPAN, (g + 1) * ISPAN)
                accs = [
                    pacc.tile([P, OUT + 1], F32, tag=f"acc{m}", name=f"acc{m}")
                    for m in range(OCG)
                ]
                for q in range(NQ):
                    stage2_quad(g, gsl, q, accs)
                epilogue(g, accs)

    if split_waits:
        split_excess_waits(nc)
    return nc


# ---------------------------------------------------------------------------
# host wrapper
# ---------------------------------------------------------------------------
_CACHE = {}


def _get_nc(NJ, R, IN, OUT):
    key = (NJ, R, IN, OUT)
    if key not in _CACHE:
        _CACHE[key] = build_nc(NJ, R, IN, OUT)
    return _CACHE[key]


def _tile_inpT(inp_bf16, P=128):
    """[Nrows, IN] -> [128, (Nrows/P)*IN] where chunk (jc,kc) at cols
    jc*IN+kc*P..+P holds inpT[kc*P+p, jc*P+t] = inp[jc*P+t, kc*P+p]."""
    Nr, IN = inp_bf16.shape
    JC, KC = Nr // P, IN // P
    a = inp_bf16.reshape(JC, P, KC, P)        # [jc, t, kc, p]
    a = np.ascontiguousarray(a.transpose(3, 0, 2, 1))  # [p, jc, kc, t]
    return a.reshape(P, JC * IN)


def prep_in_maps(inp, adj, W, a1, a2, n_cores=N_CORES):
    """Host-side layout prep: slicing + transposition + dtype casts only."""
    N, IN = inp.shape
    OUT = W.shape[1]
    R = N // n_cores
    bf16 = ml_dtypes.bfloat16
    inp_bf = inp.astype(bf16)
    W_bf = np.ascontiguousarray(W.astype(bf16))
    WT = np.ascontiguousarray(W.T).astype(bf16)
    a12 = np.ascontiguousarray(np.concatenate([a1, a2], axis=1)).astype(bf16)
    adjT = np.ascontiguousarray(adj.T).astype(np.int8)
    f8 = ml_dtypes.float8_e4m3
    ident = np.eye(128, dtype=np.float32)

    def _tile_mask(m8, QW=2):
        # [NJ, R] -> [128, G*NQ*QW*ISPAN]; tile (g, q) contiguous [b, i]
        P, ISPAN = 128, 768
        NQ, G = N // (QW * P), R // ISPAN
        a = m8.reshape(NQ, QW, P, G, ISPAN)        # [q, b, p, g, i]
        a = np.ascontiguousarray(a.transpose(2, 3, 0, 1, 4))  # [p,g,q,b,i]
        return a.reshape(P, G * NQ * QW * ISPAN)

    in_maps = []
    for c in range(n_cores):
        sl = slice(c * R, (c + 1) * R)
        # roll j so chunks 0..OC-1 are this core's own rows
        jorder = np.roll(np.arange(N), -c * R)
        a8 = np.ascontiguousarray(adjT[jorder][:, sl])
        in_maps.append({
            "mcast": _tile_mask(a8.astype(f8)),
            "madd": _tile_mask(((a8 - 1) * 96).astype(np.float32).astype(f8)),
            "inpT_t": _tile_inpT(np.ascontiguousarray(inp_bf[jorder])),
            "W": W_bf,
            "WT": WT,
            "a12": a12,
            "ident": ident,
        })
    return in_maps, R, IN, OUT


def kernel(inp, adj, W, a1, a2):
    inp = np.asarray(inp, dtype=np.float32)
    adj = np.asarray(adj, dtype=np.int32)
    W = np.asarray(W, dtype=np.float32)
    a1 = np.asarray(a1, dtype=np.float32)
    a2 = np.asarray(a2, dtype=np.float32)
    N = inp.shape[0]
    in_maps, R, IN, OUT = prep_in_maps(inp, adj, W, a1, a2)
    nc = _get_nc(N, R, IN, OUT)
    res = run_bass_kernel_spmd(nc, in_maps, list(range(N_CORES)))
    return np.concatenate(
        [res.results[c]["out"] for c in range(N_CORES)], axis=0
    )
